# revision 1
# baseline (speedup 1.0000x reference)
"""Transformer kernel builder for TRN2 (Bass/Tile), data-parallel over batch.

Per-core: 2 batch elements (T=1024 tokens), full weights.
Feature-major activations [D, T]; f32r matmuls; fp16 FFN hidden + W2.
"""
import numpy as np
from contextlib import ExitStack

import concourse.bass as bass
import concourse.bacc as bacc
import concourse.tile as tile
from concourse import mybir
from concourse.masks import make_identity

P = 128
S = 512
BL = 2            # local batches per core
T = S * BL        # 1024 tokens per core
D = 1024
H = 16
DK = 64
DHID = 4096
DOUT = 10000
L = 4
LN_EPS = 1e-5
MASK_RATE = 0.15
NDC = D // P      # 8 d-chunks
NHC = DHID // P   # 32 hid chunks
NOC = 20          # dout chunks of 512 (last 272)

f32 = mybir.dt.float32
f32r = mybir.dt.float32r
f16 = mybir.dt.float16
AF = mybir.ActivationFunctionType
OP = mybir.AluOpType

UW = 2048         # unit width in fp32 elements (8 KiB slots)


_name_ctr = [0]


def _nm(prefix):
    _name_ctr[0] += 1
    return f"{prefix}{_name_ctr[0]}"


class FM:
    """Chunked buffer: nch chunks of [128, ncols], packed into 8 KiB units."""

    def __init__(self, pool, nch, ncols, dtype):
        self.nch, self.ncols = nch, ncols
        uw = UW * (2 if dtype == f16 else 1)
        self.cpu = max(1, uw // ncols)
        n_units = (nch + self.cpu - 1) // self.cpu
        self.units = [pool.tile([P, self.cpu * ncols], dtype, tag="u",
                                name=_nm("fm"))
                      for _ in range(n_units)]

    def sl(self, dc, c0=0, n=None, p0=0, np_=P):
        n = self.ncols - c0 if n is None else n
        u = self.units[dc // self.cpu]
        base = (dc % self.cpu) * self.ncols
        return u[p0:p0 + np_, base + c0: base + c0 + n]


def build(n_layers=L, do_final=True, dumps=(), n_cores=8, u_bufs=16):
    nc = bacc.Bacc("TRN2", target_bir_lowering=False, debug=False,
                   num_devices=n_cores)
    dp = nc.declare_dram_parameter
    xb = dp("xb", [S, BL, D], f32, isOutput=False)
    rnd = dp("rnd", [BL, S], f32, isOutput=False)
    posi = dp("posi", [S, D], f32, isOutput=False)
    ln0w = dp("ln0_w", [D], f32, isOutput=False)
    ln0b = dp("ln0_b", [D], f32, isOutput=False)
    WqT = dp("WqT", [L, D, D], f32r, isOutput=False)
    WkT = dp("WkT", [L, D, D], f32r, isOutput=False)
    WvT = dp("WvT", [L, D, D], f32r, isOutput=False)
    WfcT = dp("WfcT", [L, D, D], f32r, isOutput=False)
    W1T = dp("W1T", [L, D, DHID], f32r, isOutput=False)
    W2T = dp("W2T", [L, DHID, D], f16, isOutput=False)
    bqc = dp("bqc", [L, P, NDC], f32, isOutput=False)
    bkc = dp("bkc", [L, P, NDC], f32, isOutput=False)
    bfcc = dp("bfcc", [L, P, NDC], f32, isOutput=False)
    b1c = dp("b1c", [L, P, NHC], f32, isOutput=False)
    b2c = dp("b2c", [L, P, NDC], f32, isOutput=False)
    l1wc = dp("l1wc", [L, P, NDC], f32, isOutput=False)
    l1bc = dp("l1bc", [L, P, NDC], f32, isOutput=False)
    l2wc = dp("l2wc", [L, P, NDC], f32, isOutput=False)
    l2bc = dp("l2bc", [L, P, NDC], f32, isOutput=False)
    bv = dp("bv", [L, D], f32, isOutput=False)
    WoT = dp("WoT", [D, DOUT], f32r, isOutput=False)
    bo = dp("bo", [DOUT], f32, isOutput=False)
    out = dp("out", [S, BL, DOUT], f32, isOutput=True) if do_final else None
    dump_t = {}

    def dump_fm(nm, fm):
        if nm not in dumps:
            return
        w = fm.units[0].shape[1]
        dt_ = fm.units[0].dtype
        dump_t[nm] = dp("dump_" + nm, [len(fm.units), P, w], dt_, isOutput=True)
        for i, u in enumerate(fm.units):
            nc.sync.dma_start(dump_t[nm][i], u[:])

    with tile.TileContext(nc) as tc:
        with ExitStack() as ctx:
            ctx.enter_context(nc.allow_low_precision(
                "f32r/f16 matmul operands by design; accumulation is f32"))
            pu = ctx.enter_context(tc.tile_pool(name="pu", bufs=u_bufs))
            pw = ctx.enter_context(tc.tile_pool(name="pw", bufs=6))
            pwl = ctx.enter_context(tc.tile_pool(name="pwl", bufs=8))
            pwr = ctx.enter_context(tc.tile_pool(name="pwr", bufs=4))
            pb = ctx.enter_context(tc.tile_pool(name="pb", bufs=10))
            pbv = ctx.enter_context(tc.tile_pool(name="pbv", bufs=1))
            pr = ctx.enter_context(tc.tile_pool(name="pr", bufs=6))
            pst = ctx.enter_context(tc.tile_pool(name="pst", bufs=8))
            pc = ctx.enter_context(tc.tile_pool(name="pc", bufs=1))
            ps = ctx.enter_context(tc.tile_pool(name="ps", bufs=8, space="PSUM"))

            # ---- constants ----
            ident = pc.tile([P, P], f32, tag="c_id")
            make_identity(nc, ident[:])
            ones_f = pc.tile([P, 1], f32, tag="c_of")
            nc.vector.memset(ones_f[:], 1.0)
            ones_col = pc.tile([P, 1], f32r, tag="c_oc")
            nc.vector.tensor_copy(ones_col[:], ones_f[:])
            ones_rf = pc.tile([1, P], f32, tag="c_orf")
            nc.vector.memset(ones_rf[:], 1.0)
            ones_row = pc.tile([1, P], f32r, tag="c_or")
            nc.vector.tensor_copy(ones_row[:], ones_rf[:])
            lnw_rep = pc.tile([P, D], f32, tag="c_lnw")
            nc.sync.dma_start(lnw_rep[:], ln0w[:].rearrange("(o d) -> o d", o=1)
                              .to_broadcast((P, D)))
            lnb_rep = pc.tile([P, D], f32, tag="c_lnb")
            eps_col = pc.tile([P, 1], f32, tag="c_eps")
            nc.vector.memset(eps_col[:], LN_EPS)
            nc.sync.dma_start(lnb_rep[:], ln0b[:].rearrange("(o d) -> o d", o=1)
                              .to_broadcast((P, D)))

            def psum(shape=(P, 512), dtype=f32):
                return ps.tile(list(shape), dtype, tag="ps", name=_nm("ps"))

            # ================= embed =================
            posib = FM(pu, 4, 1024, f32)  # s-chunk major
            for scj in range(4):
                pt = posib.sl(scj)
                nc.sync.dma_start(pt, posi[scj * P:(scj + 1) * P, :])
                nc.vector.tensor_tensor(out=pt, in0=pt, in1=lnb_rep[:], op=OP.add)

            enc = FM(pu, NDC, 1024, f32r)
            for b in range(BL):
                for sc in range(4):
                    x_tm = pw.tile([P, D], f32, tag="w")
                    nc.sync.dma_start(x_tm[:], xb[sc * P:(sc + 1) * P, b, :])
                    kcol = pst.tile([P, 1], f32, tag="st")
                    nc.sync.dma_start(
                        kcol[:], rnd[b, sc * P:(sc + 1) * P]
                        .rearrange("(p o) -> p o", o=1))
                    km = pst.tile([P, 1], f32, tag="st")
                    nc.vector.tensor_scalar(out=km[:], in0=kcol[:],
                                            scalar1=MASK_RATE, scalar2=None,
                                            op0=OP.is_gt)
                    h = pw.tile([P, D], f32, tag="w")
                    nc.vector.tensor_scalar(out=h[:], in0=x_tm[:], scalar1=km[:],
                                            scalar2=None, op0=OP.mult)
                    stats = pst.tile([P, 2, 6], f32, tag="st6")
                    hr = h[:].rearrange("p (g f) -> p g f", g=2)
                    for g in range(2):
                        nc.vector.bn_stats(out=stats[:, g, :], in_=hr[:, g, :])
                    mv = pst.tile([P, 2], f32, tag="st")
                    nc.vector.bn_aggr(out=mv[:], in_=stats[:])
                    sd = pst.tile([P, 1], f32, tag="st")
                    nc.scalar.activation(out=sd[:], in_=mv[:, 1:2], func=AF.Sqrt,
                                         bias=eps_col[:])
                    rs = pst.tile([P, 1], f32, tag="st")
                    nc.vector.reciprocal(out=rs[:], in_=sd[:])
                    t1 = pw.tile([P, D], f32, tag="w")
                    nc.vector.scalar_tensor_tensor(
                        out=t1[:], in0=h[:], scalar=mv[:, 0:1],
                        in1=rs[:].to_broadcast((P, D)),
                        op0=OP.subtract, op1=OP.mult)
                    nc.vector.tensor_tensor(out=t1[:], in0=t1[:], in1=lnw_rep[:],
                                            op=OP.mult)
                    nc.vector.tensor_tensor(out=t1[:], in0=t1[:],
                                            in1=posib.sl(sc), op=OP.add)
                    for j in range(NDC):
                        pt = psum((P, P))
                        nc.tensor.transpose(pt[:, :], t1[:, j * P:(j + 1) * P],
                                            ident[:])
                        nc.vector.tensor_copy(
                            enc.sl(j, b * S + sc * P, P), pt[:, :])

            dump_fm("enc0", enc)

            # ================= helpers =================
            def load_cols(src, l, n):
                t = pb.tile([P, n], f32, tag="b")
                nc.sync.dma_start(t[:], src[l])
                return t

            def ln_fm(X, c0_in, n, w_t, b_t, dst, c0_out):
                """LN over the feature (partition-chunk) dim on columns
                [c0_in, c0_in+n) of X, writing [c0_out, c0_out+n) of dst."""
                mu_ps = psum((1, n))
                sq_ps = psum((1, n))
                for dc in range(NDC):
                    xs = X.sl(dc, c0_in, n)
                    nc.tensor.matmul(mu_ps[:, :], ones_col[:], xs,
                                     start=(dc == 0), stop=(dc == NDC - 1))
                    sq = pw.tile([P, n], f32r, tag="w")
                    nc.vector.tensor_tensor(out=sq[:], in0=xs, in1=xs, op=OP.mult)
                    nc.tensor.matmul(sq_ps[:, :], ones_col[:], sq[:],
                                     start=(dc == 0), stop=(dc == NDC - 1))
                mu = pr.tile([1, n], f32, tag="r")
                nc.vector.tensor_scalar(out=mu[:], in0=mu_ps[:, :],
                                        scalar1=1.0 / D, scalar2=None, op0=OP.mult)
                mu2 = pr.tile([1, n], f32, tag="r")
                nc.vector.tensor_tensor(out=mu2[:], in0=mu[:], in1=mu[:],
                                        op=OP.mult)
                var = pr.tile([1, n], f32, tag="r")
                nc.vector.scalar_tensor_tensor(
                    out=var[:], in0=sq_ps[:, :], scalar=1.0 / D, in1=mu2[:],
                    op0=OP.mult, op1=OP.subtract)
                nc.scalar.activation(out=var[:], in_=var[:], func=AF.Sqrt,
                                     bias=eps_col[0:1, :])
                g_r = pr.tile([1, n], f32r, tag="r")
                nc.vector.reciprocal(out=g_r[:], in_=var[:])
                c_r = pr.tile([1, n], f32r, tag="r")
                nc.vector.tensor_tensor(out=c_r[:], in0=mu[:], in1=g_r[:],
                                        op=OP.mult)
                g_ps = psum((P, n))
                nc.tensor.matmul(g_ps[:, :], ones_row[:], g_r[:],
                                 start=True, stop=True)
                c_ps = psum((P, n))
                nc.tensor.matmul(c_ps[:, :], ones_row[:], c_r[:],
                                 start=True, stop=True)
                for dc in range(NDC):
                    t1 = pw.tile([P, n], f32, tag="w")
                    nc.vector.tensor_tensor(out=t1[:], in0=X.sl(dc, c0_in, n),
                                            in1=g_ps[:, :], op=OP.mult)
                    nc.vector.tensor_tensor(out=t1[:], in0=t1[:],
                                            in1=c_ps[:, :], op=OP.subtract)
                    nc.vector.scalar_tensor_tensor(
                        out=dst.sl(dc, c0_out, n), in0=t1[:],
                        scalar=w_t[:, dc:dc + 1],
                        in1=b_t[:, dc:dc + 1].to_broadcast((P, n)),
                        op0=OP.mult, op1=OP.add)

            # ================= layers =================
            for l in range(n_layers):
                bqt = load_cols(bqc, l, NDC)
                bkt = load_cols(bkc, l, NDC)
                bfct = load_cols(bfcc, l, NDC)
                b1t = load_cols(b1c, l, NHC)
                b2t = load_cols(b2c, l, NDC)
                l1wt = load_cols(l1wc, l, NDC)
                l1bt = load_cols(l1bc, l, NDC)
                l2wt = load_cols(l2wc, l, NDC)
                l2bt = load_cols(l2bc, l, NDC)
                bvrep = pbv.tile([P, D], f32, tag="bv")
                nc.sync.dma_start(bvrep[:], bv[l].rearrange("(o d) -> o d", o=1)
                                  .to_broadcast((P, D)))

                # ---- q/k projections, both batches, weights once ----
                qk = {}
                for (nm, W, bt) in (("q", WqT, bqt), ("k", WkT, bkt)):
                    for b in range(BL):
                        qk[(nm, b)] = FM(pu, NDC, S, f32r)
                    for m in range(NDC):
                        pps = [psum(), psum()]
                        for k in range(NDC):
                            wt = pwl.tile([P, P], f32r, tag="wl")
                            nc.sync.dma_start(
                                wt[:], W[l, k * P:(k + 1) * P, m * P:(m + 1) * P])
                            for b in range(BL):
                                nc.tensor.matmul(
                                    pps[b][:, :], wt[:], enc.sl(k, b * S, S),
                                    start=(k == 0), stop=(k == NDC - 1))
                        for b in range(BL):
                            nc.scalar.activation(
                                out=qk[(nm, b)].sl(m), in_=pps[b][:, :],
                                func=AF.Identity, bias=bt[:, m:m + 1])

                if l == 0:
                    dump_fm("q0", qk[("q", 0)])
                    dump_fm("k0", qk[("k", 0)])

                C = [None, None]
                for b in range(BL):
                    qb, kb = qk[("q", b)], qk[("k", b)]
                    # ---- v projection (token-major) for batch b ----
                    vT = FM(pu, 4, 1024, f32r)  # [512 tok, 1024 feat]
                    for n in range(2):
                        pps = [psum() for _ in range(4)]
                        for k in range(NDC):
                            wt = pwr.tile([P, 512], f32r, tag="wr")
                            nc.sync.dma_start(
                                wt[:], WvT[l, k * P:(k + 1) * P,
                                           n * 512:(n + 1) * 512])
                            for tcc in range(4):
                                nc.tensor.matmul(
                                    pps[tcc][:, :],
                                    enc.sl(k, b * S + tcc * P, P), wt[:],
                                    start=(k == 0), stop=(k == NDC - 1))
                        for tcc in range(4):
                            nc.vector.tensor_tensor(
                                out=vT.sl(tcc, n * 512, 512),
                                in0=pps[tcc][:, :],
                                in1=bvrep[:, n * 512:(n + 1) * 512], op=OP.add)
                    if l == 0 and b == 0:
                        dump_fm("v0", vT)
                    # ---- attention for batch b ----
                    att = FM(pu, NDC, S, f32r)
                    for h in range(H):
                        dc = h // 2
                        po = (h % 2) * DK
                        exps = []
                        den_ps = psum((1, S))
                        for kc in range(4):
                            sc_ps = psum()
                            nc.tensor.matmul(
                                sc_ps[:, :],
                                kb.sl(dc, kc * P, P, p0=po, np_=DK),
                                qb.sl(dc, 0, S, p0=po, np_=DK),
                                start=True, stop=True)
                            ex = pw.tile([P, S], f32r, tag="w")
                            nc.scalar.activation(out=ex[:], in_=sc_ps[:, :],
                                                 func=AF.Exp, scale=0.125)
                            exps.append(ex)
                            nc.tensor.matmul(den_ps[:, :], ones_col[:], ex[:],
                                             start=(kc == 0), stop=(kc == 3))
                        av_ps = psum((DK, S))
                        for kc in range(4):
                            nc.tensor.matmul(
                                av_ps[:, :], vT.sl(kc, h * DK, DK), exps[kc][:],
                                start=(kc == 0), stop=(kc == 3))
                        den_r = pr.tile([1, S], f32r, tag="r")
                        nc.vector.reciprocal(out=den_r[:], in_=den_ps[:, :])
                        rep_ps = psum((DK, S))
                        nc.tensor.matmul(rep_ps[:, :], ones_row[:, 0:DK],
                                         den_r[:], start=True, stop=True)
                        asl = att.sl(dc, 0, S, p0=po, np_=DK)
                        nc.vector.tensor_copy(asl, av_ps[:, :])
                        nc.vector.tensor_tensor(out=asl, in0=asl,
                                                in1=rep_ps[:, :], op=OP.mult)
                    if l == 0 and b == 0:
                        dump_fm("att0", att)
                    # ---- fc + bias + residual for batch b ----
                    C[b] = FM(pu, NDC, S, f32r)
                    for m in range(NDC):
                        pp = psum()
                        for k in range(NDC):
                            wt = pwl.tile([P, P], f32r, tag="wl")
                            nc.sync.dma_start(
                                wt[:], WfcT[l, k * P:(k + 1) * P,
                                            m * P:(m + 1) * P])
                            nc.tensor.matmul(pp[:, :], wt[:], att.sl(k),
                                             start=(k == 0), stop=(k == NDC - 1))
                        nc.vector.scalar_tensor_tensor(
                            out=C[b].sl(m), in0=pp[:, :],
                            scalar=bfct[:, m:m + 1], in1=enc.sl(m, b * S, S),
                            op0=OP.add, op1=OP.add)
                if l == 0:
                    dump_fm("c0", C[0])

                # ---- LN1 -> Dm ----
                Dm = FM(pu, NDC, 1024, f32r)
                for b in range(BL):
                    ln_fm(C[b], 0, S, l1wt, l1bt, Dm, b * S)

                # ---- FFN (both halves, weights once) ----
                hid = FM(pu, NHC, 1024, f16)
                for m in range(NHC):
                    pps = [psum(), psum()]
                    for k in range(NDC):
                        wt = pwl.tile([P, P], f32r, tag="wl")
                        nc.sync.dma_start(
                            wt[:], W1T[l, k * P:(k + 1) * P, m * P:(m + 1) * P])
                        for th in range(2):
                            nc.tensor.matmul(
                                pps[th][:, :], wt[:], Dm.sl(k, th * S, S),
                                start=(k == 0), stop=(k == NDC - 1))
                    for th in range(2):
                        nc.scalar.activation(
                            out=hid.sl(m, th * S, S), in_=pps[th][:, :],
                            func=AF.Relu, bias=b1t[:, m:m + 1])
                E = FM(pu, NDC, 1024, f32r)
                for m in range(NDC):
                    pps = [psum(), psum()]
                    for k in range(NHC):
                        wt = pwl.tile([P, P], f16, tag="wl")
                        nc.sync.dma_start(
                            wt[:], W2T[l, k * P:(k + 1) * P, m * P:(m + 1) * P])
                        for th in range(2):
                            nc.tensor.matmul(
                                pps[th][:, :], wt[:], hid.sl(k, th * S, S),
                                start=(k == 0), stop=(k == NHC - 1))
                    for th in range(2):
                        nc.vector.scalar_tensor_tensor(
                            out=E.sl(m, th * S, S), in0=pps[th][:, :],
                            scalar=b2t[:, m:m + 1], in1=Dm.sl(m, th * S, S),
                            op0=OP.add, op1=OP.add)

                # ---- LN2 -> next enc ----
                F = FM(pu, NDC, 1024, f32r)
                for th in range(2):
                    ln_fm(E, th * S, S, l2wt, l2bt, F, th * S)
                enc = F
                dump_fm(f"enc_l{l}", enc)

            # ================= final projection + log_softmax =================
            if do_final:
                for tg in range(2):
                    lgs = [[pu.tile([P, 4096], f16, tag="u", name=_nm("lg")) for _ in range(3)]
                           for _ in range(4)]
                    zaccs = [pst.tile([P, NOC], f32, tag="z", name=_nm("za")) for _ in range(4)]
                    for n in range(NOC):
                        ncols = 512 if n < NOC - 1 else DOUT - (NOC - 1) * 512
                        borep = pw.tile([P, 512], f32, tag="w")
                        nc.sync.dma_start(
                            borep[:, :ncols],
                            bo[n * 512:n * 512 + ncols]
                            .rearrange("(o d) -> o d", o=1)
                            .to_broadcast((P, ncols)))
                        pps = [psum() for _ in range(4)]
                        for k in range(NDC):
                            wt = pwr.tile([P, 512], f32r, tag="wr")
                            nc.sync.dma_start(
                                wt[:, :ncols],
                                WoT[k * P:(k + 1) * P, n * 512:n * 512 + ncols])
                            for tcc in range(4):
                                nc.tensor.matmul(
                                    pps[tcc][:, :ncols],
                                    enc.sl(k, tg * S + tcc * P, P),
                                    wt[:, :ncols],
                                    start=(k == 0), stop=(k == NDC - 1))
                        for tcc in range(4):
                            lsl = lgs[tcc][n // 8][:, (n % 8) * 512:
                                                   (n % 8) * 512 + ncols]
                            nc.vector.tensor_tensor(out=lsl, in0=pps[tcc][:, :ncols],
                                                    in1=borep[:, :ncols], op=OP.add)
                            exs = pw.tile([P, 512], f16, tag="w")
                            nc.scalar.activation(
                                out=exs[:, :ncols], in_=lsl, func=AF.Exp,
                                accum_out=zaccs[tcc][:, n:n + 1])
                    for tcc in range(4):
                        z = pst.tile([P, 1], f32, tag="st")
                        nc.vector.reduce_sum(z[:], zaccs[tcc][:],
                                             axis=mybir.AxisListType.X)
                        lz = pst.tile([P, 1], f32, tag="st")
                        nc.scalar.activation(out=lz[:], in_=z[:], func=AF.Ln)
                        for n in range(NOC):
                            ncols = 512 if n < NOC - 1 else DOUT - (NOC - 1) * 512
                            lsl = lgs[tcc][n // 8][:, (n % 8) * 512:
                                                   (n % 8) * 512 + ncols]
                            st = pw.tile([P, 512], f32, tag="w")
                            nc.vector.tensor_scalar(
                                out=st[:, :ncols], in0=lsl, scalar1=lz[:],
                                scalar2=None, op0=OP.subtract)
                            s0 = tcc * P
                            nc.sync.dma_start(
                                out[s0:s0 + P, tg, n * 512:n * 512 + ncols],
                                st[:, :ncols])
    nc.finalize()
    return nc


# ======================= host-side input prep =======================
def make_in_map(inp, core):
    """Build the per-core input dict from the full-problem input dict."""
    f = np.float32
    c = np.ascontiguousarray
    b0 = core * BL
    m = {
        "xb": c(np.asarray(inp["x"], f)[:, b0:b0 + BL, :]),
        "rnd": c(np.asarray(inp["rnd"], f)[b0:b0 + BL, :]),
        "posi": c(np.asarray(inp["posi"], f)),
        "ln0_w": c(np.asarray(inp["ln0_w"], f)),
        "ln0_b": c(np.asarray(inp["ln0_b"], f)),
        "bv": c(np.asarray(inp["bv"], f)),
        "bo": c(np.asarray(inp["bo"], f)),
    }
    tr = lambda a: c(np.asarray(a, f).transpose(0, 2, 1))
    m["WqT"] = tr(inp["Wq"])
    m["WkT"] = tr(inp["Wk"])
    m["WvT"] = tr(inp["Wv"])
    m["WfcT"] = tr(inp["Wfc"])
    m["W1T"] = tr(inp["W1"])
    m["W2T"] = np.asarray(inp["W2"], f).transpose(0, 2, 1).astype(np.float16)
    m["W2T"] = c(m["W2T"])
    m["WoT"] = c(np.asarray(inp["Wo"], f).T)
    cols = lambda a, nch: c(np.asarray(a, f).reshape(L, nch, P).transpose(0, 2, 1))
    m["bqc"] = cols(inp["bq"], NDC)
    m["bkc"] = cols(inp["bk"], NDC)
    m["bfcc"] = cols(inp["bfc"], NDC)
    m["b1c"] = cols(inp["b1"], NHC)
    m["b2c"] = cols(inp["b2"], NDC)
    m["l1wc"] = cols(inp["ln1_w"], NDC)
    m["l1bc"] = cols(inp["ln1_b"], NDC)
    m["l2wc"] = cols(inp["ln2_w"], NDC)
    m["l2bc"] = cols(inp["ln2_b"], NDC)
    return m


def fm_to_np(arr, nch, ncols, dtype_bytes=4):
    """[n_units, 128, unit_cols] -> [nch*128, ncols]."""
    n_units = arr.shape[0]
    uw = arr.shape[2]
    cpu = uw // ncols
    out = np.zeros((nch * P, ncols), arr.dtype)
    for dc in range(nch):
        u = arr[dc // cpu]
        base = (dc % cpu) * ncols
        out[dc * P:(dc + 1) * P, :] = u[:, base:base + ncols]
    return out


# ======================= entry point =======================
_NC_CACHE = {}


def _get_nc(n_cores=8):
    if n_cores not in _NC_CACHE:
        _NC_CACHE[n_cores] = build(n_layers=L, do_final=True, dumps=(),
                                   n_cores=n_cores)
    return _NC_CACHE[n_cores]


def kernel(**inputs):
    """Full-input, full-output entry point. Shards batch across 8 cores."""
    from concourse.bass_utils import run_bass_kernel_spmd
    n_cores = 8
    nc = _get_nc(n_cores)
    inp = {k: np.asarray(v) for k, v in inputs.items()}
    in_maps = [make_in_map(inp, c) for c in range(n_cores)]
    res = run_bass_kernel_spmd(nc, in_maps, list(range(n_cores)))
    outs = [res.results[c]["out"] for c in range(n_cores)]
    return np.concatenate(outs, axis=1).astype(np.float32)

